# revision 38
# baseline (speedup 1.0000x reference)
"""Trainium2 Bass kernel for the ClassificationNCA problem.

Self-contained: callable as kernel(**inputs) with the full (unsharded)
inputs; shards batch across 8 NeuronCores (2 images/core), runs the
20-step NCA entirely in SBUF, returns softmax(mean-pooled class channels).

Layout (per core): state lives in SBUF as [128, 34, 68] fp32 where
partition = img*64 + half*32 + ch: 29 channels, a constant-1.0 row at
ch 29 (carries bias+fire into the L1 matmul), 2 pad rows.  Each
(img, half) block is a 32-row half-image strip with one halo row on the
inner edge and a zero pad row on the outer edge; cols 2..65 interior.

Per step:
  - DVE sobel chain (shared-smooth factorization, images merged, split
    into 4 row-block parts that pipeline against the previous step's
    state updates):  P=(1+zv)s, C=(1+zh)P, R=(1+zv)C, S2=(1+zh)C,
    sx=R(r-1,x)-R(r-1,x-1), sy=S2(r,x-1)-S2(r-1,x-1).
  - Pool masks: s16m = S*fire (bf16), SXYm = SXY*fire (fp8).  The fire
    mask is folded into the L1 rhs (incl. the ones-row -> bias*fire), so
    masked pixels produce h1=0 -> h2=0 -> dx=0 with no separate multiply.
  - PE per (rb, combo) tile: bf16 s-tap (K=30 incl. per-step bias row,
    PE row-tiled at the combo's partitions) + fp8 DoubleRow xy-tap
    (ktiles = sx/sy planes) accumulate h1; one fp8 DoubleRow L2; bf16
    L3 column-tiled by combo so one DVE add applies dx for all 4 combos.
  - Activations: h1 on Act ([128,1024] LeakyReLU -> fp8); h2 LeakyReLU
    split between Act and a 2-op DVE form for engine balance.
"""
import sys

sys.path.insert(0, "/opt/trn_rl_repo")

import json
import numpy as np
import ml_dtypes

NUM_IMG, NUM_HID, NUM_OUT = 3, 16, 10
NCH = 29            # total channels
HIDDEN = 128
B, H, W = 16, 64, 64
N_CORES = 8
B_LOC = B // N_CORES          # images per core
WP = W + 4                    # padded width: cols 2..65 interior
SR = 34                       # strip rows: halo/pad + 32 interior + halo/pad
CB = 32                       # channel-block partition stride
CL = 2                        # interior column offset
RPT = 8                       # rows per tile
NPIX = RPT * W                # 512
NT = 16                       # tiles per step (4 rb x 4 combos)

_MAX_WAITS = 1

# per-step tile slots (j*4+c) whose h2 LeakyReLU runs as a 2-op DVE form
# instead of on Act.  Tuned for engine balance: Act carries all h1
# activations, so roughly half the h2 work moves to DVE.
_H2_DVE_SLOTS = frozenset({2, 6, 10})
_H2_DVE_FORM = "copy_stt"
_ORDER = "012"
_ACT_BUFS = 3
_SXYM_FIRST_ON_DVE = True
_ROT = 1


def _fix_bir_waits(bir_bytes: bytes) -> bytes:
    """walrus codegen allows only one embedded sem-wait per instruction;
    Tile sometimes attaches more (e.g. the kernel-tail drain).  Move excess
    waits onto NoOp carrier instructions inserted before the offender on the
    same engine."""
    bir = json.loads(bir_bytes)
    uid = 0
    for fn in bir["functions"]:
        for blk in fn["blocks"]:
            out = []
            for ins in blk["instructions"]:
                si = ins.get("sync_info")
                waits = (si or {}).get("on_wait") or []
                if len(waits) > _MAX_WAITS:
                    excess = waits[:-_MAX_WAITS]
                    si["on_wait"] = waits[-_MAX_WAITS:]
                    for i in range(0, len(excess), _MAX_WAITS):
                        out.append({
                            "opcode": "NoOp",
                            "name": f"wsplit_{uid}",
                            "engine": ins["engine"],
                            "ins": [],
                            "outs": [],
                            "sync_info": {
                                "on_wait": excess[i:i + _MAX_WAITS],
                                "on_update": [],
                            },
                        })
                        uid += 1
                out.append(ins)
            blk["instructions"] = out
    return json.dumps(bir).encode()


def _host_rng(steps: int):
    """Reproduce the reference's jax threefry random draws exactly (on CPU)."""
    import jax
    cpu = jax.devices("cpu")[0]
    with jax.default_device(cpu):
        import jax.numpy as jnp
        base = jax.random.key(42)
        hid = 0.5 + 0.225 * jax.random.normal(
            jax.random.fold_in(base, 10_000), (B, NUM_HID, H, W),
            dtype=jnp.float32)
        hid = np.asarray(hid)
        fires = np.zeros((max(steps, 1), B, H, W), np.float32)
        for s in range(steps):
            u = jax.random.uniform(jax.random.fold_in(base, s), (B, H, W, 1),
                                   dtype=jnp.float32)
            fires[s] = np.asarray(u < 0.5, np.float32)[..., 0]
    return hid, fires


def _build(steps: int):
    from concourse import mybir
    from concourse.bass import Bass
    from concourse.tile import TileContext

    f32 = mybir.dt.float32
    bf16 = mybir.dt.bfloat16
    fp8 = mybir.dt.float8e4
    LR = mybir.ActivationFunctionType.Lrelu
    DRM = mybir.MatmulPerfMode.DoubleRow
    ADD = mybir.AluOpType.add
    SUB = mybir.AluOpType.subtract
    MAX = mybir.AluOpType.max

    nc = Bass(trn_type="TRN2", target_bir_lowering=False)

    ST = max(steps, 1)
    s0_d = nc.dram_tensor("s0", [128, SR, WP], f32, kind="ExternalInput")
    fireS_d = nc.dram_tensor("fireS", [ST, 128, 32, W], bf16,
                             kind="ExternalInput")
    w1sqb_d = nc.dram_tensor("w1sqb", [128, ST, 2, HIDDEN], bf16,
                             kind="ExternalInput")
    w1xy8_d = nc.dram_tensor("w1xy8", [128, 2, 2, HIDDEN], fp8,
                             kind="ExternalInput")
    w2pk_d = nc.dram_tensor("w2pk", [128, 2, HIDDEN], fp8,
                            kind="ExternalInput")
    w3tp_d = nc.dram_tensor("w3tp", [HIDDEN, CB], bf16, kind="ExternalInput")
    out_d = nc.dram_tensor("sout", [128, SR, WP], f32, kind="ExternalOutput")

    with TileContext(nc) as tc:
        with tc.tile_pool(name="state", bufs=1) as statep, \
             tc.tile_pool(name="wts", bufs=1) as wtsp, \
             tc.tile_pool(name="chain", bufs=1) as chainp, \
             tc.tile_pool(name="mask", bufs=2) as maskp, \
             tc.tile_pool(name="fire", bufs=2) as firep, \
             tc.tile_pool(name="acts", bufs=_ACT_BUFS) as actp, \
             tc.tile_pool(name="ph1", bufs=2, space="PSUM") as ph1, \
             tc.tile_pool(name="ph2", bufs=2, space="PSUM") as ph2, \
             tc.tile_pool(name="pdx", bufs=2, space="PSUM") as pdx:

            S = [statep.tile([128, SR, WP], f32, name=f"S{k}") for k in range(2)]
            # sobel chain intermediates (bf16) and outputs (sx/sy planes)
            TP = chainp.tile([128, SR, WP], bf16, name="TP")
            TC = chainp.tile([128, SR, WP], bf16, name="TC")
            TR = chainp.tile([128, SR, WP], bf16, name="TR")
            TS = chainp.tile([128, SR, WP], bf16, name="TS")
            SXY = chainp.tile([128, 2, SR, WP], bf16, name="SXY")

            w1sqb = wtsp.tile([128, ST, 2, HIDDEN], bf16, name="w1sqb")
            w1xy8 = wtsp.tile([128, 2, 2, HIDDEN], fp8, name="w1xy8")
            w2pk = wtsp.tile([128, 2, HIDDEN], fp8, name="w2pk")
            w3tp = wtsp.tile([HIDDEN, CB], bf16, name="w3tp")

            fts = {}

            def get_ft(g):
                if g not in fts:
                    ftt = firep.tile([128, 32, W], bf16, name="ft")
                    nc.sync.dma_start(out=ftt[:], in_=fireS_d[g])
                    fts[g] = ftt
                return fts[g]

            # startup DMA order: step-0 first-rowblock deps land first so
            # the prologue chain/masks/s-taps start as early as possible
            nc.sync.dma_start(out=S[0][:, 0:11], in_=s0_d[:, 0:11])
            get_ft(0)
            nc.sync.dma_start(out=w1sqb[:, 0:1], in_=w1sqb_d[:, 0:1])
            nc.sync.dma_start(out=w1xy8[:], in_=w1xy8_d[:])
            nc.sync.dma_start(out=w2pk[:], in_=w2pk_d[:])
            nc.sync.dma_start(out=w3tp[:], in_=w3tp_d[:])
            nc.sync.dma_start(out=S[0][:, 11:SR], in_=s0_d[:, 11:SR])
            if ST > 1:
                nc.sync.dma_start(out=w1sqb[:, 1:ST], in_=w1sqb_d[:, 1:ST])
            nc.sync.dma_start(out=S[1][:], in_=s0_d[:])

            # chain part covering sx/sy strip rows a..b (1-based interior)
            # fast_first=(xn, ft): reorder as P,C,S2,sy,R,sx and emit the
            # sy-plane mask on Pool right after sy so it overlaps the DVE
            # x-path; the sx-plane mask then runs on DVE (critical path).
            def emit_chain_part(g, a, b, fast_first=None):
                """sx/sy for interior strip rows a..b (inclusive, 1-based).
                P=(1+zv)S rows a-1..b; C=(1+zh)P same rows; R=(1+zv)C rows
                a-1..b-1; S2=(1+zh)C rows a-1..b;
                sx(r,x)=R(r-1,x)-R(r-1,x-1); sy(r,x)=S2(r,x-1)-S2(r-1,x-1)."""
                cur = S[g % 2]
                pa = a - 1
                nc.vector.tensor_tensor(
                    out=TP[:, pa:b + 1, 1:67], op=ADD,
                    in0=cur[:, pa:b + 1, 1:67], in1=cur[:, pa + 1:b + 2, 1:67])
                nc.vector.tensor_tensor(
                    out=TC[:, pa:b + 1, 1:66], op=ADD,
                    in0=TP[:, pa:b + 1, 1:66], in1=TP[:, pa:b + 1, 2:67])

                def y_path():
                    nc.vector.tensor_tensor(
                        out=TS[:, pa:b + 1, 1:65], op=ADD,
                        in0=TC[:, pa:b + 1, 1:65], in1=TC[:, pa:b + 1, 2:66])
                    nc.vector.tensor_tensor(
                        out=SXY[:, 1, a:b + 1, CL:CL + W], op=SUB,
                        in0=TS[:, a:b + 1, 1:65], in1=TS[:, pa:b, 1:65])

                def x_path():
                    nc.vector.tensor_tensor(
                        out=TR[:, pa:b, 1:66], op=ADD,
                        in0=TC[:, pa:b, 1:66], in1=TC[:, a:b + 1, 1:66])
                    nc.vector.tensor_tensor(
                        out=SXY[:, 0, a:b + 1, CL:CL + W], op=SUB,
                        in0=TR[:, pa:b, CL:CL + W], in1=TR[:, pa:b, 1:65])

                if fast_first is None:
                    x_path()
                    y_path()
                    return
                xn, ft = fast_first
                y_path()
                nc.gpsimd.tensor_mul(
                    out=xn[:, 1, pa:b, :],
                    in0=SXY[:, 1, a:b + 1, CL:CL + W],
                    in1=ft[:, pa:b, :])
                x_path()
                nc.vector.tensor_mul(
                    out=xn[:, 0, pa:b, :],
                    in0=SXY[:, 0, a:b + 1, CL:CL + W],
                    in1=ft[:, pa:b, :])

            def emit_masks(g, rb, s16m, sxym, ft):
                """Pool: masked bf16 shadow + masked fp8 sobel planes for
                one row-block (interior rows rb*8+1 .. rb*8+8)."""
                cur = S[g % 2]
                r0 = rb * RPT
                nc.gpsimd.tensor_mul(
                    out=s16m[:, r0:r0 + RPT, :],
                    in0=cur[:, r0 + 1:r0 + RPT + 1, CL:CL + W],
                    in1=ft[:, r0:r0 + RPT, :])
                for pl in range(2):
                    nc.gpsimd.tensor_mul(
                        out=sxym[:, pl, r0:r0 + RPT, :],
                        in0=SXY[:, pl, r0 + 1:r0 + RPT + 1, CL:CL + W],
                        in1=ft[:, r0:r0 + RPT, :])

            def emit_halos(g):
                nxt = S[(g + 1) % 2]
                for i in range(B_LOC):
                    ip = i * 2 * CB
                    nc.gpsimd.tensor_copy(out=nxt[ip + CB:ip + CB + NCH, 0:1, :],
                                          in_=nxt[ip:ip + NCH, 32:33, :])
                    nc.gpsimd.tensor_copy(out=nxt[ip:ip + NCH, 33:34, :],
                                          in_=nxt[ip + CB:ip + CB + NCH, 1:2, :])

            # --- steady-state software pipeline over (step, rb, combo) -----
            # rb processing order rotates by +1 each step so each step's
            # chain part (needing dx of rbs r-1..r+1 of the previous step)
            # is ready exactly when the PE reaches it.
            t_ord = [[(g * _ROT + j) % 4 for j in range(4)] for g in range(steps)]

            def tile_at(idx):
                """Global tile index -> (g, rb, c)."""
                g, r = divmod(idx, NT)
                j, c = divmod(r, 4)
                return g, t_ord[g][j], c

            # per-step tiles for masks (double buffered via pool)
            s16m_t, sxym_t = {}, {}
            chain_done = {}

            def get_masks(g):
                if g not in s16m_t:
                    s16m_t[g] = maskp.tile([128, 32, W], bf16, name="s16m")
                    sxym_t[g] = maskp.tile([128, 2, 32, W], fp8, name="sxym")
                return s16m_t[g], sxym_t[g]

            # warm the PE p-state during the init DMAs
            for _w in range(24):
                wp = ph1.tile([HIDDEN, 2, NPIX], f32, name="h1")
                nc.tensor.matmul(wp[:, 0, 0:HIDDEN], w1sqb[0:HIDDEN, 0, 0, :],
                                 w1sqb[0:HIDDEN, 0, 0, :], start=True, stop=True)

            # prologue for step 0: first rb's chain+masks, then the rest as
            # one span so the PE can start while the span computes
            m0, x0 = get_masks(0)
            rb0 = t_ord[0][0]
            r0, r1 = rb0 * RPT, rb0 * RPT + RPT
            nc.gpsimd.tensor_mul(
                out=m0[:, r0:r1, :],
                in0=S[0][:, r0 + 1:r1 + 1, CL:CL + W],
                in1=get_ft(0)[:, r0:r1, :])
            emit_chain_part(0, rb0 * RPT + 1, rb0 * RPT + RPT,
                            fast_first=(x0, get_ft(0)))
            for rb in (r for r in t_ord[0] if r != rb0):
                emit_chain_part(0, rb * RPT + 1, rb * RPT + RPT)
                emit_masks(0, rb, m0, x0, get_ft(0))

            TOT = steps * NT
            pend_h1 = {}     # idx -> (h1 psum, h1s8)
            pend_h2 = {}     # idx -> (h2 psum, h2s)
            pend_dx = {}     # rb-group key -> psum tile

            def emit_stage0(idx):
                g, rb, c = tile_at(idx)
                s16m, sxym = get_masks(g)
                p0 = c * CB
                r0 = rb * RPT
                h1 = ph1.tile([HIDDEN, 2, NPIX], f32, name="h1")
                for half in range(2):
                    nc.tensor.matmul(
                        h1[:, half], w1sqb[p0:p0 + NCH + 1, g, half, :],
                        s16m[p0:p0 + NCH + 1, r0:r0 + RPT, :],
                        start=True, stop=False, tile_position=(p0, 0))
                    nc.tensor.matmul(
                        h1[:, half], w1xy8[p0:p0 + CB, :, half, :],
                        sxym[p0:p0 + CB, :, r0:r0 + RPT, :],
                        start=False, stop=True, perf_mode=DRM,
                        tile_position=(p0, 0))
                pend_h1[idx] = h1

            def emit_stage1(idx):
                h1 = pend_h1.pop(idx)
                h1s8 = actp.tile([HIDDEN, 2, NPIX], fp8, name="h1s8")
                nc.scalar.activation(
                    out=h1s8[:, :, :].rearrange("p a b -> p (a b)"),
                    in_=h1[:, :, :].rearrange("p a b -> p (a b)"),
                    func=LR, bias=0.0, scale=1.0, alpha=0.01)
                h2 = ph2.tile([HIDDEN, NPIX], f32, name="h2")
                nc.tensor.matmul(h2[:], w2pk[:], h1s8[:], start=True, stop=True,
                                 perf_mode=DRM)
                pend_h2[idx] = h2

            def pos_of(g, rb):
                return t_ord[g].index(rb)

            def emit_stage2(idx):
                g, rb, c = tile_at(idx)
                h2 = pend_h2.pop(idx)
                h2s = actp.tile([HIDDEN, NPIX], bf16, name="h2s")
                if (idx % NT) not in _H2_DVE_SLOTS:
                    nc.scalar.activation(out=h2s[:], in_=h2[:], func=LR,
                                         bias=0.0, scale=1.0, alpha=0.01)
                elif _H2_DVE_FORM == "copy_stt":
                    # psum->bf16 copy, then all-SBUF stt max(x, 0.01x) which
                    # runs in the DVE 2x mode
                    tq = actp.tile([HIDDEN, NPIX], bf16, name="tq")
                    nc.vector.tensor_copy(out=tq[:], in_=h2[:])
                    nc.vector.scalar_tensor_tensor(
                        out=h2s[:], in0=tq[:], scalar=0.01, in1=tq[:],
                        op0=mybir.AluOpType.mult, op1=MAX)
                else:
                    tq = actp.tile([HIDDEN, NPIX], bf16, name="tq")
                    nc.vector.tensor_scalar_mul(out=tq[:], in0=h2[:],
                                                scalar1=0.01)
                    nc.vector.tensor_tensor(out=h2s[:], in0=tq[:], in1=h2[:],
                                            op=MAX)
                key = (g, rb)
                if key not in pend_dx:
                    pend_dx[key] = pdx.tile([128, RPT, W], f32, name="dx")
                dxp = pend_dx[key]
                nc.tensor.matmul(
                    dxp[p0b(c):p0b(c) + CB, :, :].rearrange("p a b -> p (a b)"),
                    w3tp[:], h2s[:], start=True, stop=True,
                    tile_position=(0, p0b(c)))
                if c != 3:
                    return
                cur, nxt = S[g % 2], S[(g + 1) % 2]
                r0 = rb * RPT
                nc.vector.tensor_tensor(
                    out=nxt[:, r0 + 1:r0 + RPT + 1, CL:CL + W], op=ADD,
                    in0=cur[:, r0 + 1:r0 + RPT + 1, CL:CL + W], in1=dxp[:])
                del pend_dx[(g, rb)]
                if g == steps - 1:
                    # ship this row-block as soon as it is final; the
                    # epilogue only reads interior rows 1..32, so halo/pad
                    # rows are never transferred
                    nc.sync.dma_start(
                        out=out_d[:, r0 + 1:r0 + RPT + 1, :],
                        in_=nxt[:, r0 + 1:r0 + RPT + 1, :])
                # emit next-step chain/mask parts whose state-row deps are
                # now satisfied: the masks/chain for step g+1 MUST be emitted
                # (program order) before step g+1's matmuls read them, and
                # after their own dx-add deps; chain part rb' reads S rows
                # 8rb'..8rb'+9 -> needs dx of rb'-1, rb', rb'+1 and halos for
                # the edge parts (halos themselves need dx3+dx0).
                jpos = pos_of(g, rb)
                hpos = max(pos_of(g, 0), pos_of(g, 3))
                if jpos == hpos:
                    emit_halos(g)
                if g + 1 >= steps:
                    return
                gn = g + 1
                mn, xn = get_masks(gn)

                # s16m parts only need their own rb's dx and run on the
                # (otherwise idle) Pool engine, so emit them as soon as
                # ready.  Chain parts (DVE) are emitted once their row
                # neighbours' dx (and halos for edge parts) have landed —
                # the first-consumed part becomes ready one dx-add early,
                # shortening the next step's restart; the rest follow after
                # the last dx-add in consumption order.
                for r in range(4):
                    if pos_of(g, r) == jpos:
                        r0, r1 = r * RPT, r * RPT + RPT
                        nc.gpsimd.tensor_mul(
                            out=mn[:, r0:r1, :],
                            in0=S[gn % 2][:, r0 + 1:r1 + 1, CL:CL + W],
                            in1=get_ft(gn)[:, r0:r1, :])

                def chain_pos(rbn):
                    need = [rbn] + [r for r in (rbn - 1, rbn + 1) if 0 <= r < 4]
                    cp = max(pos_of(g, r) for r in need)
                    if rbn in (0, 3):
                        cp = max(cp, hpos)
                    return cp

                # NOTE: the first-consumed part MUST be emitted at its
                # readiness site (jpos<=2): the emission stagger runs the
                # next step's first two stage0s before stage2 of this
                # step's last tile, so waiting for jpos==3 would emit the
                # mask writers after their readers (= race on hardware).
                done = chain_done.setdefault(gn, set())
                for r in t_ord[gn]:
                    if r in done or chain_pos(r) > jpos:
                        continue
                    done.add(r)
                    if _SXYM_FIRST_ON_DVE and r == t_ord[gn][0]:
                        emit_chain_part(gn, r * RPT + 1, r * RPT + RPT,
                                        fast_first=(xn, get_ft(gn)))
                        continue
                    emit_chain_part(gn, r * RPT + 1, r * RPT + RPT)
                    r0, r1 = r * RPT, r * RPT + RPT
                    for pl in range(2):
                        nc.gpsimd.tensor_mul(
                            out=xn[:, pl, r0:r1, :],
                            in0=SXY[:, pl, r0 + 1:r1 + 1, CL:CL + W],
                            in1=get_ft(gn)[:, r0:r1, :])

            for idx in range(TOT + 2):
                # stage2 first: the last tiles' h2-act/L3/dx must enter the
                # in-order Act/PE queues before the next step's first-tile
                # work, or the engines head-block on not-yet-ready ops.
                if _ORDER == "210":
                    if 2 <= idx and idx - 2 < TOT:
                        emit_stage2(idx - 2)
                    if 1 <= idx and idx - 1 < TOT:
                        emit_stage1(idx - 1)
                    if idx < TOT:
                        emit_stage0(idx)
                elif _ORDER == "120":
                    if 1 <= idx and idx - 1 < TOT:
                        emit_stage1(idx - 1)
                    if 2 <= idx and idx - 2 < TOT:
                        emit_stage2(idx - 2)
                    if idx < TOT:
                        emit_stage0(idx)
                else:
                    if idx < TOT:
                        emit_stage0(idx)
                    if 1 <= idx and idx - 1 < TOT:
                        emit_stage1(idx - 1)
                    if 2 <= idx and idx - 2 < TOT:
                        emit_stage2(idx - 2)

            # output rows shipped per-rb inside the last step's dx-adds

    orig = nc.to_json_bytes
    nc.to_json_bytes = lambda: _fix_bir_waits(orig())
    return nc


def p0b(c):
    return c * CB


_CACHE = {}


def _get_nc(steps: int):
    if steps not in _CACHE:
        _CACHE[steps] = _build(steps)
    return _CACHE[steps]


def _prep_inputs(x, w1, b1, w2, w3, steps):
    """Host-side input preparation; returns per-core input maps."""
    x = np.asarray(x, np.float32)
    w1 = np.asarray(w1, np.float32)
    b1 = np.asarray(b1, np.float32)
    w2 = np.asarray(w2, np.float32)
    w3 = np.asarray(w3, np.float32)

    hid, fires = _host_rng(steps)
    ST = max(steps, 1)

    bf = ml_dtypes.bfloat16
    f8 = ml_dtypes.float8_e4m3fn

    # full padded state [B, 32, 66, 68]; ones-row at channel slot 29
    state0 = np.zeros((B, CB, H + 2, WP), np.float32)
    state0[:, :NUM_IMG, 1:1 + H, CL:CL + W] = x
    state0[:, NUM_IMG:NUM_IMG + NUM_HID, 1:1 + H, CL:CL + W] = hid
    state0[:, NCH, :, :] = 1.0

    # w1sqb: s-tap weights + per-step (bias + temporal) row, [128, ST, 2, 128]
    w1sqb = np.zeros((128, ST, 2, HIDDEN), np.float32)
    for b0 in (0, 32, 64, 96):
        for half in range(2):
            cs = slice(half * HIDDEN, (half + 1) * HIDDEN)
            w1sqb[b0:b0 + NCH, :, half, :] = w1[cs, 0:NCH].T[:, None, :]
            for t in range(steps):
                be = b1[cs] + w1[cs, 3 * NCH] * (np.float32(t) / np.float32(100.0))
                w1sqb[b0 + NCH, t, half, :] = be
    w1sqb = w1sqb.astype(bf)

    # w1xy8: [128, ktile(sx/sy), half, 128] fp8, /8 sobel scale folded in
    w1xy8 = np.zeros((128, 2, 2, HIDDEN), np.float32)
    for b0 in (0, 32, 64, 96):
        for half in range(2):
            cs = slice(half * HIDDEN, (half + 1) * HIDDEN)
            w1xy8[b0:b0 + NCH, 0, half, :] = w1[cs, NCH:2 * NCH].T / 8.0
            w1xy8[b0:b0 + NCH, 1, half, :] = w1[cs, 2 * NCH:3 * NCH].T / 8.0
    w1xy8 = w1xy8.astype(f8)

    # w2pk: [p, ktile, m] with ktile j holding hidden dims j*128+p
    w2pk = np.zeros((128, 2, HIDDEN), np.float32)
    for j in range(2):
        w2pk[:, j, :] = w2[:, j * HIDDEN:(j + 1) * HIDDEN].T
    w2pk = w2pk.astype(f8)

    # w3tp: [128, 32] with cols 29..31 zero; image channels immutable
    w3tp = np.zeros((HIDDEN, CB), np.float32)
    w3tp[:, :NCH] = w3.T
    w3tp[:, :NUM_IMG] = 0.0
    w3tp = w3tp.astype(bf)

    in_maps = []
    for cidx in range(N_CORES):
        imgs = slice(cidx * B_LOC, (cidx + 1) * B_LOC)
        sc = state0[imgs]                      # [B_LOC, 32, 66, 68]
        s0 = np.stack([sc[:, :, 0:SR, :], sc[:, :, 32:32 + SR, :]], axis=1)
        s0 = s0.reshape(B_LOC * 2 * CB, SR, WP)
        # fireS in strip layout: partition p -> (img, half) block's fire rows
        f = fires[:ST, imgs]                   # [ST, B_LOC, H, W]
        fS = np.zeros((ST, 128, 32, W), np.float32)
        for i in range(B_LOC):
            for hf in range(2):
                blk = f[:, i, hf * 32:hf * 32 + 32, :]      # [ST, 32, W]
                p0 = i * 2 * CB + hf * CB
                fS[:, p0:p0 + CB] = blk[:, None]
        in_maps.append({
            "s0": np.ascontiguousarray(s0),
            "fireS": np.ascontiguousarray(fS).astype(bf),
            "w1sqb": w1sqb, "w1xy8": w1xy8,
            "w2pk": w2pk, "w3tp": w3tp,
        })
    return in_maps


def _softmax(x):
    m = x.max(axis=-1, keepdims=True)
    e = np.exp(x - m)
    return e / e.sum(axis=-1, keepdims=True)


def _epilogue(results):
    logits = np.zeros((B, NUM_OUT), np.float32)
    for c, res in enumerate(results):
        so = res["sout"].reshape(B_LOC, 2, CB, SR, WP)
        cls = so[:, :, NUM_IMG + NUM_HID:NCH, 1:33, CL:CL + W]
        logits[c * B_LOC:(c + 1) * B_LOC] = cls.mean(axis=(1, 3, 4))
    return _softmax(logits).astype(np.float32)


def _run(trace=False, _in_maps=None, **inputs):
    from concourse.bass_utils import run_bass_kernel_spmd
    steps = int(inputs["steps"])
    if steps == 0:
        return _softmax(np.zeros((B, NUM_OUT), np.float32)), None
    in_maps = _in_maps
    if in_maps is None:
        in_maps = _prep_inputs(inputs["x"], inputs["w1"], inputs["b1"],
                               inputs["w2"], inputs["w3"], steps)
    nc = _get_nc(steps)
    r = run_bass_kernel_spmd(nc, in_maps, core_ids=list(range(N_CORES)),
                             trace=trace)
    return _epilogue(r.results), r.exec_time_ns


def predicted_exec_ns(steps: int = 20) -> float:
    """Cost-model (TimelineSim) estimate of on-device execution time for the
    whole job (all cores run the same program in parallel)."""
    from concourse.timeline_sim import TimelineSim
    nc = _build(int(steps))
    return TimelineSim(nc, trace=False).simulate()


def kernel(**inputs) -> np.ndarray:
    out, _ = _run(trace=False, **inputs)
    return out


# revision 40
# speedup vs baseline: 1.0250x; 1.0250x over previous
"""Trainium2 Bass kernel for the ClassificationNCA problem.

Self-contained: callable as kernel(**inputs) with the full (unsharded)
inputs; shards batch across 8 NeuronCores (2 images/core), runs the
20-step NCA entirely in SBUF, returns softmax(mean-pooled class channels).

Layout (per core): state lives in SBUF as [128, 34, 68] fp32 where
partition = img*64 + half*32 + ch: 29 channels, a constant-1.0 row at
ch 29 (carries bias+fire into the L1 matmul), 2 pad rows.  Each
(img, half) block is a 32-row half-image strip with one halo row on the
inner edge and a zero pad row on the outer edge; cols 2..65 interior.

Per step:
  - DVE sobel chain (shared-smooth factorization, images merged, split
    into 4 row-block parts that pipeline against the previous step's
    state updates):  P=(1+zv)s, C=(1+zh)P, R=(1+zv)C, S2=(1+zh)C,
    sx=R(r-1,x)-R(r-1,x-1), sy=S2(r,x-1)-S2(r-1,x-1).
  - Pool masks: s16m = S*fire (bf16), SXYm = SXY*fire (fp8).  The fire
    mask is folded into the L1 rhs (incl. the ones-row -> bias*fire), so
    masked pixels produce h1=0 -> h2=0 -> dx=0 with no separate multiply.
  - PE per (rb, combo) tile: bf16 s-tap (K=30 incl. per-step bias row,
    PE row-tiled at the combo's partitions) + fp8 DoubleRow xy-tap
    (ktiles = sx/sy planes) accumulate h1; one fp8 DoubleRow L2; bf16
    L3 column-tiled by combo so one DVE add applies dx for all 4 combos.
  - Activations: h1 on Act ([128,1024] LeakyReLU -> fp8); h2 LeakyReLU
    split between Act and a 2-op DVE form for engine balance.
"""
import sys

sys.path.insert(0, "/opt/trn_rl_repo")

import json
import numpy as np
import ml_dtypes

NUM_IMG, NUM_HID, NUM_OUT = 3, 16, 10
NCH = 29            # total channels
HIDDEN = 128
B, H, W = 16, 64, 64
N_CORES = 8
B_LOC = B // N_CORES          # images per core
WP = W + 4                    # padded width: cols 2..65 interior
SR = 34                       # strip rows: halo/pad + 32 interior + halo/pad
CB = 32                       # channel-block partition stride
CL = 2                        # interior column offset
RPT = 8                       # rows per tile
NPIX = RPT * W                # 512
NT = 16                       # tiles per step (4 rb x 4 combos)

_MAX_WAITS = 1

# per-step tile slots (j*4+c) whose h2 LeakyReLU runs as a 2-op DVE form
# instead of on Act.  Tuned for engine balance: Act carries all h1
# activations, so roughly half the h2 work moves to DVE.
_H2_DVE_SLOTS = frozenset({2, 6, 10})
_H2_DVE_FORM = "copy_stt"
_ORDER = "012"
_ACT_BUFS = 3
_SXYM_FIRST_ON_DVE = True
_NFAST = 2
_ROT = 1


def _fix_bir_waits(bir_bytes: bytes) -> bytes:
    """walrus codegen allows only one embedded sem-wait per instruction;
    Tile sometimes attaches more (e.g. the kernel-tail drain).  Move excess
    waits onto NoOp carrier instructions inserted before the offender on the
    same engine."""
    bir = json.loads(bir_bytes)
    uid = 0
    for fn in bir["functions"]:
        for blk in fn["blocks"]:
            out = []
            for ins in blk["instructions"]:
                si = ins.get("sync_info")
                waits = (si or {}).get("on_wait") or []
                if len(waits) > _MAX_WAITS:
                    excess = waits[:-_MAX_WAITS]
                    si["on_wait"] = waits[-_MAX_WAITS:]
                    for i in range(0, len(excess), _MAX_WAITS):
                        out.append({
                            "opcode": "NoOp",
                            "name": f"wsplit_{uid}",
                            "engine": ins["engine"],
                            "ins": [],
                            "outs": [],
                            "sync_info": {
                                "on_wait": excess[i:i + _MAX_WAITS],
                                "on_update": [],
                            },
                        })
                        uid += 1
                out.append(ins)
            blk["instructions"] = out
    return json.dumps(bir).encode()


def _host_rng(steps: int):
    """Reproduce the reference's jax threefry random draws exactly (on CPU)."""
    import jax
    cpu = jax.devices("cpu")[0]
    with jax.default_device(cpu):
        import jax.numpy as jnp
        base = jax.random.key(42)
        hid = 0.5 + 0.225 * jax.random.normal(
            jax.random.fold_in(base, 10_000), (B, NUM_HID, H, W),
            dtype=jnp.float32)
        hid = np.asarray(hid)
        fires = np.zeros((max(steps, 1), B, H, W), np.float32)
        for s in range(steps):
            u = jax.random.uniform(jax.random.fold_in(base, s), (B, H, W, 1),
                                   dtype=jnp.float32)
            fires[s] = np.asarray(u < 0.5, np.float32)[..., 0]
    return hid, fires


def _build(steps: int):
    from concourse import mybir
    from concourse.bass import Bass
    from concourse.tile import TileContext

    f32 = mybir.dt.float32
    bf16 = mybir.dt.bfloat16
    fp8 = mybir.dt.float8e4
    LR = mybir.ActivationFunctionType.Lrelu
    DRM = mybir.MatmulPerfMode.DoubleRow
    ADD = mybir.AluOpType.add
    SUB = mybir.AluOpType.subtract
    MAX = mybir.AluOpType.max

    nc = Bass(trn_type="TRN2", target_bir_lowering=False)

    ST = max(steps, 1)
    s0_d = nc.dram_tensor("s0", [128, SR, WP], f32, kind="ExternalInput")
    fireS_d = nc.dram_tensor("fireS", [ST, 128, 32, W], bf16,
                             kind="ExternalInput")
    w1sqb_d = nc.dram_tensor("w1sqb", [128, ST, 2, HIDDEN], bf16,
                             kind="ExternalInput")
    w1xy8_d = nc.dram_tensor("w1xy8", [128, 2, 2, HIDDEN], fp8,
                             kind="ExternalInput")
    w2pk_d = nc.dram_tensor("w2pk", [128, 2, HIDDEN], fp8,
                            kind="ExternalInput")
    w3tp_d = nc.dram_tensor("w3tp", [HIDDEN, CB], bf16, kind="ExternalInput")
    out_d = nc.dram_tensor("sout", [128, SR, WP], f32, kind="ExternalOutput")

    with TileContext(nc) as tc:
        with tc.tile_pool(name="state", bufs=1) as statep, \
             tc.tile_pool(name="wts", bufs=1) as wtsp, \
             tc.tile_pool(name="chain", bufs=1) as chainp, \
             tc.tile_pool(name="mask", bufs=2) as maskp, \
             tc.tile_pool(name="fire", bufs=2) as firep, \
             tc.tile_pool(name="acts", bufs=_ACT_BUFS) as actp, \
             tc.tile_pool(name="ph1", bufs=2, space="PSUM") as ph1, \
             tc.tile_pool(name="ph2", bufs=2, space="PSUM") as ph2, \
             tc.tile_pool(name="pdx", bufs=2, space="PSUM") as pdx:

            S = [statep.tile([128, SR, WP], f32, name=f"S{k}") for k in range(2)]
            # sobel chain intermediates (bf16) and outputs (sx/sy planes)
            TP = chainp.tile([128, SR, WP], bf16, name="TP")
            TC = chainp.tile([128, SR, WP], bf16, name="TC")
            TR = chainp.tile([128, SR, WP], bf16, name="TR")
            TS = chainp.tile([128, SR, WP], bf16, name="TS")
            SXY = chainp.tile([128, 2, SR, WP], bf16, name="SXY")

            w1sqb = wtsp.tile([128, ST, 2, HIDDEN], bf16, name="w1sqb")
            w1xy8 = wtsp.tile([128, 2, 2, HIDDEN], fp8, name="w1xy8")
            w2pk = wtsp.tile([128, 2, HIDDEN], fp8, name="w2pk")
            w3tp = wtsp.tile([HIDDEN, CB], bf16, name="w3tp")

            fts = {}

            def get_ft(g):
                if g not in fts:
                    ftt = firep.tile([128, 32, W], bf16, name="ft")
                    nc.sync.dma_start(out=ftt[:], in_=fireS_d[g])
                    fts[g] = ftt
                return fts[g]

            # startup DMA order: step-0 first-rowblock deps land first so
            # the prologue chain/masks/s-taps start as early as possible
            nc.sync.dma_start(out=S[0][:, 0:11], in_=s0_d[:, 0:11])
            get_ft(0)
            nc.sync.dma_start(out=w1sqb[:, 0:1], in_=w1sqb_d[:, 0:1])
            nc.sync.dma_start(out=w1xy8[:], in_=w1xy8_d[:])
            nc.sync.dma_start(out=w2pk[:], in_=w2pk_d[:])
            nc.sync.dma_start(out=w3tp[:], in_=w3tp_d[:])
            nc.sync.dma_start(out=S[0][:, 11:SR], in_=s0_d[:, 11:SR])
            if ST > 1:
                nc.sync.dma_start(out=w1sqb[:, 1:ST], in_=w1sqb_d[:, 1:ST])
            nc.sync.dma_start(out=S[1][:], in_=s0_d[:])

            # chain part covering sx/sy strip rows a..b (1-based interior)
            # fast_first=(xn, ft): reorder as P,C,S2,sy,R,sx and emit the
            # sy-plane mask on Pool right after sy so it overlaps the DVE
            # x-path; the sx-plane mask then runs on DVE (critical path).
            def emit_chain_part(g, a, b, fast_first=None):
                """sx/sy for interior strip rows a..b (inclusive, 1-based).
                P=(1+zv)S rows a-1..b; C=(1+zh)P same rows; R=(1+zv)C rows
                a-1..b-1; S2=(1+zh)C rows a-1..b;
                sx(r,x)=R(r-1,x)-R(r-1,x-1); sy(r,x)=S2(r,x-1)-S2(r-1,x-1)."""
                cur = S[g % 2]
                pa = a - 1
                nc.vector.tensor_tensor(
                    out=TP[:, pa:b + 1, 1:67], op=ADD,
                    in0=cur[:, pa:b + 1, 1:67], in1=cur[:, pa + 1:b + 2, 1:67])
                nc.vector.tensor_tensor(
                    out=TC[:, pa:b + 1, 1:66], op=ADD,
                    in0=TP[:, pa:b + 1, 1:66], in1=TP[:, pa:b + 1, 2:67])

                def y_path():
                    nc.vector.tensor_tensor(
                        out=TS[:, pa:b + 1, 1:65], op=ADD,
                        in0=TC[:, pa:b + 1, 1:65], in1=TC[:, pa:b + 1, 2:66])
                    nc.vector.tensor_tensor(
                        out=SXY[:, 1, a:b + 1, CL:CL + W], op=SUB,
                        in0=TS[:, a:b + 1, 1:65], in1=TS[:, pa:b, 1:65])

                def x_path():
                    nc.vector.tensor_tensor(
                        out=TR[:, pa:b, 1:66], op=ADD,
                        in0=TC[:, pa:b, 1:66], in1=TC[:, a:b + 1, 1:66])
                    nc.vector.tensor_tensor(
                        out=SXY[:, 0, a:b + 1, CL:CL + W], op=SUB,
                        in0=TR[:, pa:b, CL:CL + W], in1=TR[:, pa:b, 1:65])

                if fast_first is None:
                    x_path()
                    y_path()
                    return
                xn, ft = fast_first
                y_path()
                nc.gpsimd.tensor_mul(
                    out=xn[:, 1, pa:b, :],
                    in0=SXY[:, 1, a:b + 1, CL:CL + W],
                    in1=ft[:, pa:b, :])
                x_path()
                nc.vector.tensor_mul(
                    out=xn[:, 0, pa:b, :],
                    in0=SXY[:, 0, a:b + 1, CL:CL + W],
                    in1=ft[:, pa:b, :])

            def emit_masks(g, rb, s16m, sxym, ft):
                """Pool: masked bf16 shadow + masked fp8 sobel planes for
                one row-block (interior rows rb*8+1 .. rb*8+8)."""
                cur = S[g % 2]
                r0 = rb * RPT
                nc.gpsimd.tensor_mul(
                    out=s16m[:, r0:r0 + RPT, :],
                    in0=cur[:, r0 + 1:r0 + RPT + 1, CL:CL + W],
                    in1=ft[:, r0:r0 + RPT, :])
                for pl in range(2):
                    nc.gpsimd.tensor_mul(
                        out=sxym[:, pl, r0:r0 + RPT, :],
                        in0=SXY[:, pl, r0 + 1:r0 + RPT + 1, CL:CL + W],
                        in1=ft[:, r0:r0 + RPT, :])

            def emit_halos(g):
                nxt = S[(g + 1) % 2]
                for i in range(B_LOC):
                    ip = i * 2 * CB
                    nc.gpsimd.tensor_copy(out=nxt[ip + CB:ip + CB + NCH, 0:1, :],
                                          in_=nxt[ip:ip + NCH, 32:33, :])
                    nc.gpsimd.tensor_copy(out=nxt[ip:ip + NCH, 33:34, :],
                                          in_=nxt[ip + CB:ip + CB + NCH, 1:2, :])

            # --- steady-state software pipeline over (step, rb, combo) -----
            # rb processing order rotates by +1 each step so each step's
            # chain part (needing dx of rbs r-1..r+1 of the previous step)
            # is ready exactly when the PE reaches it.
            t_ord = [[(g * _ROT + j) % 4 for j in range(4)] for g in range(steps)]

            def tile_at(idx):
                """Global tile index -> (g, rb, c)."""
                g, r = divmod(idx, NT)
                j, c = divmod(r, 4)
                return g, t_ord[g][j], c

            # per-step tiles for masks (double buffered via pool)
            s16m_t, sxym_t = {}, {}
            chain_done = {}

            def get_masks(g):
                if g not in s16m_t:
                    s16m_t[g] = maskp.tile([128, 32, W], bf16, name="s16m")
                    sxym_t[g] = maskp.tile([128, 2, 32, W], fp8, name="sxym")
                return s16m_t[g], sxym_t[g]

            # warm the PE p-state during the init DMAs
            for _w in range(24):
                wp = ph1.tile([HIDDEN, 2, NPIX], f32, name="h1")
                nc.tensor.matmul(wp[:, 0, 0:HIDDEN], w1sqb[0:HIDDEN, 0, 0, :],
                                 w1sqb[0:HIDDEN, 0, 0, :], start=True, stop=True)

            # prologue for step 0: first rb's chain+masks, then the rest as
            # one span so the PE can start while the span computes
            m0, x0 = get_masks(0)
            rb0 = t_ord[0][0]
            r0, r1 = rb0 * RPT, rb0 * RPT + RPT
            nc.gpsimd.tensor_mul(
                out=m0[:, r0:r1, :],
                in0=S[0][:, r0 + 1:r1 + 1, CL:CL + W],
                in1=get_ft(0)[:, r0:r1, :])
            emit_chain_part(0, rb0 * RPT + 1, rb0 * RPT + RPT,
                            fast_first=(x0, get_ft(0)))
            for rb in (r for r in t_ord[0] if r != rb0):
                emit_chain_part(0, rb * RPT + 1, rb * RPT + RPT)
                emit_masks(0, rb, m0, x0, get_ft(0))

            TOT = steps * NT
            pend_h1 = {}     # idx -> (h1 psum, h1s8)
            pend_h2 = {}     # idx -> (h2 psum, h2s)
            pend_dx = {}     # rb-group key -> psum tile

            def emit_stage0(idx):
                g, rb, c = tile_at(idx)
                s16m, sxym = get_masks(g)
                p0 = c * CB
                r0 = rb * RPT
                h1 = ph1.tile([HIDDEN, 2, NPIX], f32, name="h1")
                for half in range(2):
                    nc.tensor.matmul(
                        h1[:, half], w1sqb[p0:p0 + NCH + 1, g, half, :],
                        s16m[p0:p0 + NCH + 1, r0:r0 + RPT, :],
                        start=True, stop=False, tile_position=(p0, 0))
                    nc.tensor.matmul(
                        h1[:, half], w1xy8[p0:p0 + CB, :, half, :],
                        sxym[p0:p0 + CB, :, r0:r0 + RPT, :],
                        start=False, stop=True, perf_mode=DRM,
                        tile_position=(p0, 0))
                pend_h1[idx] = h1

            def emit_stage1(idx):
                h1 = pend_h1.pop(idx)
                h1s8 = actp.tile([HIDDEN, 2, NPIX], fp8, name="h1s8")
                nc.scalar.activation(
                    out=h1s8[:, :, :].rearrange("p a b -> p (a b)"),
                    in_=h1[:, :, :].rearrange("p a b -> p (a b)"),
                    func=LR, bias=0.0, scale=1.0, alpha=0.01)
                h2 = ph2.tile([HIDDEN, NPIX], f32, name="h2")
                nc.tensor.matmul(h2[:], w2pk[:], h1s8[:], start=True, stop=True,
                                 perf_mode=DRM)
                pend_h2[idx] = h2

            def pos_of(g, rb):
                return t_ord[g].index(rb)

            def emit_stage2(idx):
                g, rb, c = tile_at(idx)
                h2 = pend_h2.pop(idx)
                h2s = actp.tile([HIDDEN, NPIX], bf16, name="h2s")
                if (idx % NT) not in _H2_DVE_SLOTS:
                    nc.scalar.activation(out=h2s[:], in_=h2[:], func=LR,
                                         bias=0.0, scale=1.0, alpha=0.01)
                elif _H2_DVE_FORM == "copy_stt":
                    # psum->bf16 copy, then all-SBUF stt max(x, 0.01x) which
                    # runs in the DVE 2x mode
                    tq = actp.tile([HIDDEN, NPIX], bf16, name="tq")
                    nc.vector.tensor_copy(out=tq[:], in_=h2[:])
                    nc.vector.scalar_tensor_tensor(
                        out=h2s[:], in0=tq[:], scalar=0.01, in1=tq[:],
                        op0=mybir.AluOpType.mult, op1=MAX)
                else:
                    tq = actp.tile([HIDDEN, NPIX], bf16, name="tq")
                    nc.vector.tensor_scalar_mul(out=tq[:], in0=h2[:],
                                                scalar1=0.01)
                    nc.vector.tensor_tensor(out=h2s[:], in0=tq[:], in1=h2[:],
                                            op=MAX)
                key = (g, rb)
                if key not in pend_dx:
                    pend_dx[key] = pdx.tile([128, RPT, W], f32, name="dx")
                dxp = pend_dx[key]
                nc.tensor.matmul(
                    dxp[p0b(c):p0b(c) + CB, :, :].rearrange("p a b -> p (a b)"),
                    w3tp[:], h2s[:], start=True, stop=True,
                    tile_position=(0, p0b(c)))
                if c != 3:
                    return
                cur, nxt = S[g % 2], S[(g + 1) % 2]
                r0 = rb * RPT
                nc.vector.tensor_tensor(
                    out=nxt[:, r0 + 1:r0 + RPT + 1, CL:CL + W], op=ADD,
                    in0=cur[:, r0 + 1:r0 + RPT + 1, CL:CL + W], in1=dxp[:])
                del pend_dx[(g, rb)]
                if g == steps - 1:
                    # ship this row-block as soon as it is final; the
                    # epilogue only reads interior rows 1..32, so halo/pad
                    # rows are never transferred
                    nc.sync.dma_start(
                        out=out_d[:, r0 + 1:r0 + RPT + 1, :],
                        in_=nxt[:, r0 + 1:r0 + RPT + 1, :])
                # emit next-step chain/mask parts whose state-row deps are
                # now satisfied: the masks/chain for step g+1 MUST be emitted
                # (program order) before step g+1's matmuls read them, and
                # after their own dx-add deps; chain part rb' reads S rows
                # 8rb'..8rb'+9 -> needs dx of rb'-1, rb', rb'+1 and halos for
                # the edge parts (halos themselves need dx3+dx0).
                jpos = pos_of(g, rb)
                hpos = max(pos_of(g, 0), pos_of(g, 3))
                if jpos == hpos:
                    emit_halos(g)
                if g + 1 >= steps:
                    return
                gn = g + 1
                mn, xn = get_masks(gn)

                # s16m parts only need their own rb's dx and run on the
                # (otherwise idle) Pool engine, so emit them as soon as
                # ready.  Chain parts (DVE) are emitted once their row
                # neighbours' dx (and halos for edge parts) have landed —
                # the first-consumed part becomes ready one dx-add early,
                # shortening the next step's restart; the rest follow after
                # the last dx-add in consumption order.
                for r in range(4):
                    if pos_of(g, r) == jpos:
                        r0, r1 = r * RPT, r * RPT + RPT
                        nc.gpsimd.tensor_mul(
                            out=mn[:, r0:r1, :],
                            in0=S[gn % 2][:, r0 + 1:r1 + 1, CL:CL + W],
                            in1=get_ft(gn)[:, r0:r1, :])

                def chain_pos(rbn):
                    need = [rbn] + [r for r in (rbn - 1, rbn + 1) if 0 <= r < 4]
                    cp = max(pos_of(g, r) for r in need)
                    if rbn in (0, 3):
                        cp = max(cp, hpos)
                    return cp

                # NOTE: the first-consumed part MUST be emitted at its
                # readiness site (jpos<=2): the emission stagger runs the
                # next step's first two stage0s before stage2 of this
                # step's last tile, so waiting for jpos==3 would emit the
                # mask writers after their readers (= race on hardware).
                done = chain_done.setdefault(gn, set())
                for r in t_ord[gn]:
                    if r in done or chain_pos(r) > jpos:
                        continue
                    done.add(r)
                    if _SXYM_FIRST_ON_DVE and r in t_ord[gn][:_NFAST]:
                        emit_chain_part(gn, r * RPT + 1, r * RPT + RPT,
                                        fast_first=(xn, get_ft(gn)))
                        continue
                    emit_chain_part(gn, r * RPT + 1, r * RPT + RPT)
                    r0, r1 = r * RPT, r * RPT + RPT
                    for pl in range(2):
                        nc.gpsimd.tensor_mul(
                            out=xn[:, pl, r0:r1, :],
                            in0=SXY[:, pl, r0 + 1:r1 + 1, CL:CL + W],
                            in1=get_ft(gn)[:, r0:r1, :])

            for idx in range(TOT + 2):
                # stage2 first: the last tiles' h2-act/L3/dx must enter the
                # in-order Act/PE queues before the next step's first-tile
                # work, or the engines head-block on not-yet-ready ops.
                if _ORDER == "210":
                    if 2 <= idx and idx - 2 < TOT:
                        emit_stage2(idx - 2)
                    if 1 <= idx and idx - 1 < TOT:
                        emit_stage1(idx - 1)
                    if idx < TOT:
                        emit_stage0(idx)
                elif _ORDER == "120":
                    if 1 <= idx and idx - 1 < TOT:
                        emit_stage1(idx - 1)
                    if 2 <= idx and idx - 2 < TOT:
                        emit_stage2(idx - 2)
                    if idx < TOT:
                        emit_stage0(idx)
                else:
                    if idx < TOT:
                        emit_stage0(idx)
                    if 1 <= idx and idx - 1 < TOT:
                        emit_stage1(idx - 1)
                    if 2 <= idx and idx - 2 < TOT:
                        emit_stage2(idx - 2)

            # output rows shipped per-rb inside the last step's dx-adds

    orig = nc.to_json_bytes
    nc.to_json_bytes = lambda: _fix_bir_waits(orig())
    return nc


def p0b(c):
    return c * CB


_CACHE = {}


def _get_nc(steps: int):
    if steps not in _CACHE:
        _CACHE[steps] = _build(steps)
    return _CACHE[steps]


def _prep_inputs(x, w1, b1, w2, w3, steps):
    """Host-side input preparation; returns per-core input maps."""
    x = np.asarray(x, np.float32)
    w1 = np.asarray(w1, np.float32)
    b1 = np.asarray(b1, np.float32)
    w2 = np.asarray(w2, np.float32)
    w3 = np.asarray(w3, np.float32)

    hid, fires = _host_rng(steps)
    ST = max(steps, 1)

    bf = ml_dtypes.bfloat16
    f8 = ml_dtypes.float8_e4m3fn

    # full padded state [B, 32, 66, 68]; ones-row at channel slot 29
    state0 = np.zeros((B, CB, H + 2, WP), np.float32)
    state0[:, :NUM_IMG, 1:1 + H, CL:CL + W] = x
    state0[:, NUM_IMG:NUM_IMG + NUM_HID, 1:1 + H, CL:CL + W] = hid
    state0[:, NCH, :, :] = 1.0

    # w1sqb: s-tap weights + per-step (bias + temporal) row, [128, ST, 2, 128]
    w1sqb = np.zeros((128, ST, 2, HIDDEN), np.float32)
    for b0 in (0, 32, 64, 96):
        for half in range(2):
            cs = slice(half * HIDDEN, (half + 1) * HIDDEN)
            w1sqb[b0:b0 + NCH, :, half, :] = w1[cs, 0:NCH].T[:, None, :]
            for t in range(steps):
                be = b1[cs] + w1[cs, 3 * NCH] * (np.float32(t) / np.float32(100.0))
                w1sqb[b0 + NCH, t, half, :] = be
    w1sqb = w1sqb.astype(bf)

    # w1xy8: [128, ktile(sx/sy), half, 128] fp8, /8 sobel scale folded in
    w1xy8 = np.zeros((128, 2, 2, HIDDEN), np.float32)
    for b0 in (0, 32, 64, 96):
        for half in range(2):
            cs = slice(half * HIDDEN, (half + 1) * HIDDEN)
            w1xy8[b0:b0 + NCH, 0, half, :] = w1[cs, NCH:2 * NCH].T / 8.0
            w1xy8[b0:b0 + NCH, 1, half, :] = w1[cs, 2 * NCH:3 * NCH].T / 8.0
    w1xy8 = w1xy8.astype(f8)

    # w2pk: [p, ktile, m] with ktile j holding hidden dims j*128+p
    w2pk = np.zeros((128, 2, HIDDEN), np.float32)
    for j in range(2):
        w2pk[:, j, :] = w2[:, j * HIDDEN:(j + 1) * HIDDEN].T
    w2pk = w2pk.astype(f8)

    # w3tp: [128, 32] with cols 29..31 zero; image channels immutable
    w3tp = np.zeros((HIDDEN, CB), np.float32)
    w3tp[:, :NCH] = w3.T
    w3tp[:, :NUM_IMG] = 0.0
    w3tp = w3tp.astype(bf)

    in_maps = []
    for cidx in range(N_CORES):
        imgs = slice(cidx * B_LOC, (cidx + 1) * B_LOC)
        sc = state0[imgs]                      # [B_LOC, 32, 66, 68]
        s0 = np.stack([sc[:, :, 0:SR, :], sc[:, :, 32:32 + SR, :]], axis=1)
        s0 = s0.reshape(B_LOC * 2 * CB, SR, WP)
        # fireS in strip layout: partition p -> (img, half) block's fire rows
        f = fires[:ST, imgs]                   # [ST, B_LOC, H, W]
        fS = np.zeros((ST, 128, 32, W), np.float32)
        for i in range(B_LOC):
            for hf in range(2):
                blk = f[:, i, hf * 32:hf * 32 + 32, :]      # [ST, 32, W]
                p0 = i * 2 * CB + hf * CB
                fS[:, p0:p0 + CB] = blk[:, None]
        in_maps.append({
            "s0": np.ascontiguousarray(s0),
            "fireS": np.ascontiguousarray(fS).astype(bf),
            "w1sqb": w1sqb, "w1xy8": w1xy8,
            "w2pk": w2pk, "w3tp": w3tp,
        })
    return in_maps


def _softmax(x):
    m = x.max(axis=-1, keepdims=True)
    e = np.exp(x - m)
    return e / e.sum(axis=-1, keepdims=True)


def _epilogue(results):
    logits = np.zeros((B, NUM_OUT), np.float32)
    for c, res in enumerate(results):
        so = res["sout"].reshape(B_LOC, 2, CB, SR, WP)
        cls = so[:, :, NUM_IMG + NUM_HID:NCH, 1:33, CL:CL + W]
        logits[c * B_LOC:(c + 1) * B_LOC] = cls.mean(axis=(1, 3, 4))
    return _softmax(logits).astype(np.float32)


def _run(trace=False, _in_maps=None, **inputs):
    from concourse.bass_utils import run_bass_kernel_spmd
    steps = int(inputs["steps"])
    if steps == 0:
        return _softmax(np.zeros((B, NUM_OUT), np.float32)), None
    in_maps = _in_maps
    if in_maps is None:
        in_maps = _prep_inputs(inputs["x"], inputs["w1"], inputs["b1"],
                               inputs["w2"], inputs["w3"], steps)
    nc = _get_nc(steps)
    r = run_bass_kernel_spmd(nc, in_maps, core_ids=list(range(N_CORES)),
                             trace=trace)
    return _epilogue(r.results), r.exec_time_ns


def predicted_exec_ns(steps: int = 20) -> float:
    """Cost-model (TimelineSim) estimate of on-device execution time for the
    whole job (all cores run the same program in parallel)."""
    from concourse.timeline_sim import TimelineSim
    nc = _build(int(steps))
    return TimelineSim(nc, trace=False).simulate()


def kernel(**inputs) -> np.ndarray:
    out, _ = _run(trace=False, **inputs)
    return out


# revision 41
# speedup vs baseline: 1.0363x; 1.0110x over previous
"""Trainium2 Bass kernel for the ClassificationNCA problem.

Self-contained: callable as kernel(**inputs) with the full (unsharded)
inputs; shards batch across 8 NeuronCores (2 images/core), runs the
20-step NCA entirely in SBUF, returns softmax(mean-pooled class channels).

Layout (per core): state lives in SBUF as [128, 34, 68] fp32 where
partition = img*64 + half*32 + ch: 29 channels, a constant-1.0 row at
ch 29 (carries bias+fire into the L1 matmul), 2 pad rows.  Each
(img, half) block is a 32-row half-image strip with one halo row on the
inner edge and a zero pad row on the outer edge; cols 2..65 interior.

Per step:
  - DVE sobel chain (shared-smooth factorization, images merged, split
    into 4 row-block parts that pipeline against the previous step's
    state updates):  P=(1+zv)s, C=(1+zh)P, R=(1+zv)C, S2=(1+zh)C,
    sx=R(r-1,x)-R(r-1,x-1), sy=S2(r,x-1)-S2(r-1,x-1).
  - Pool masks: s16m = S*fire (bf16), SXYm = SXY*fire (fp8).  The fire
    mask is folded into the L1 rhs (incl. the ones-row -> bias*fire), so
    masked pixels produce h1=0 -> h2=0 -> dx=0 with no separate multiply.
  - PE per (rb, combo) tile: bf16 s-tap (K=30 incl. per-step bias row,
    PE row-tiled at the combo's partitions) + fp8 DoubleRow xy-tap
    (ktiles = sx/sy planes) accumulate h1; one fp8 DoubleRow L2; bf16
    L3 column-tiled by combo so one DVE add applies dx for all 4 combos.
  - Activations: h1 on Act ([128,1024] LeakyReLU -> fp8); h2 LeakyReLU
    split between Act and a 2-op DVE form for engine balance.
"""
import sys

sys.path.insert(0, "/opt/trn_rl_repo")

import json
import numpy as np
import ml_dtypes

NUM_IMG, NUM_HID, NUM_OUT = 3, 16, 10
NCH = 29            # total channels
HIDDEN = 128
B, H, W = 16, 64, 64
N_CORES = 8
B_LOC = B // N_CORES          # images per core
WP = W + 4                    # padded width: cols 2..65 interior
SR = 34                       # strip rows: halo/pad + 32 interior + halo/pad
CB = 32                       # channel-block partition stride
CL = 2                        # interior column offset
RPT = 8                       # rows per tile
NPIX = RPT * W                # 512
NT = 16                       # tiles per step (4 rb x 4 combos)

_MAX_WAITS = 1

# per-step tile slots (j*4+c) whose h2 LeakyReLU runs as a 2-op DVE form
# instead of on Act.  Tuned for engine balance: Act carries all h1
# activations, so roughly half the h2 work moves to DVE.
_H2_DVE_SLOTS = frozenset({6, 10})
_H2_DVE_FORM = "copy_stt"
_ORDER = "012"
_ACT_BUFS = 3
_SXYM_FIRST_ON_DVE = True
_NFAST = 2
_ROT = 1


def _fix_bir_waits(bir_bytes: bytes) -> bytes:
    """walrus codegen allows only one embedded sem-wait per instruction;
    Tile sometimes attaches more (e.g. the kernel-tail drain).  Move excess
    waits onto NoOp carrier instructions inserted before the offender on the
    same engine."""
    bir = json.loads(bir_bytes)
    uid = 0
    for fn in bir["functions"]:
        for blk in fn["blocks"]:
            out = []
            for ins in blk["instructions"]:
                si = ins.get("sync_info")
                waits = (si or {}).get("on_wait") or []
                if len(waits) > _MAX_WAITS:
                    excess = waits[:-_MAX_WAITS]
                    si["on_wait"] = waits[-_MAX_WAITS:]
                    for i in range(0, len(excess), _MAX_WAITS):
                        out.append({
                            "opcode": "NoOp",
                            "name": f"wsplit_{uid}",
                            "engine": ins["engine"],
                            "ins": [],
                            "outs": [],
                            "sync_info": {
                                "on_wait": excess[i:i + _MAX_WAITS],
                                "on_update": [],
                            },
                        })
                        uid += 1
                out.append(ins)
            blk["instructions"] = out
    return json.dumps(bir).encode()


def _host_rng(steps: int):
    """Reproduce the reference's jax threefry random draws exactly (on CPU)."""
    import jax
    cpu = jax.devices("cpu")[0]
    with jax.default_device(cpu):
        import jax.numpy as jnp
        base = jax.random.key(42)
        hid = 0.5 + 0.225 * jax.random.normal(
            jax.random.fold_in(base, 10_000), (B, NUM_HID, H, W),
            dtype=jnp.float32)
        hid = np.asarray(hid)
        fires = np.zeros((max(steps, 1), B, H, W), np.float32)
        for s in range(steps):
            u = jax.random.uniform(jax.random.fold_in(base, s), (B, H, W, 1),
                                   dtype=jnp.float32)
            fires[s] = np.asarray(u < 0.5, np.float32)[..., 0]
    return hid, fires


def _build(steps: int):
    from concourse import mybir
    from concourse.bass import Bass
    from concourse.tile import TileContext

    f32 = mybir.dt.float32
    bf16 = mybir.dt.bfloat16
    fp8 = mybir.dt.float8e4
    LR = mybir.ActivationFunctionType.Lrelu
    DRM = mybir.MatmulPerfMode.DoubleRow
    ADD = mybir.AluOpType.add
    SUB = mybir.AluOpType.subtract
    MAX = mybir.AluOpType.max

    nc = Bass(trn_type="TRN2", target_bir_lowering=False)

    ST = max(steps, 1)
    s0_d = nc.dram_tensor("s0", [128, SR, WP], f32, kind="ExternalInput")
    fireS_d = nc.dram_tensor("fireS", [ST, 128, 32, W], bf16,
                             kind="ExternalInput")
    w1sqb_d = nc.dram_tensor("w1sqb", [128, ST, 2, HIDDEN], bf16,
                             kind="ExternalInput")
    w1xy8_d = nc.dram_tensor("w1xy8", [128, 2, 2, HIDDEN], fp8,
                             kind="ExternalInput")
    w2pk_d = nc.dram_tensor("w2pk", [128, 2, HIDDEN], fp8,
                            kind="ExternalInput")
    w3tp_d = nc.dram_tensor("w3tp", [HIDDEN, CB], bf16, kind="ExternalInput")
    out_d = nc.dram_tensor("sout", [128, SR, WP], f32, kind="ExternalOutput")

    with TileContext(nc) as tc:
        with tc.tile_pool(name="state", bufs=1) as statep, \
             tc.tile_pool(name="wts", bufs=1) as wtsp, \
             tc.tile_pool(name="chain", bufs=1) as chainp, \
             tc.tile_pool(name="mask", bufs=2) as maskp, \
             tc.tile_pool(name="fire", bufs=2) as firep, \
             tc.tile_pool(name="acts", bufs=_ACT_BUFS) as actp, \
             tc.tile_pool(name="ph1", bufs=2, space="PSUM") as ph1, \
             tc.tile_pool(name="ph2", bufs=2, space="PSUM") as ph2, \
             tc.tile_pool(name="pdx", bufs=2, space="PSUM") as pdx:

            S = [statep.tile([128, SR, WP], f32, name=f"S{k}") for k in range(2)]
            # sobel chain intermediates (bf16) and outputs (sx/sy planes)
            TP = chainp.tile([128, SR, WP], bf16, name="TP")
            TC = chainp.tile([128, SR, WP], bf16, name="TC")
            TR = chainp.tile([128, SR, WP], bf16, name="TR")
            TS = chainp.tile([128, SR, WP], bf16, name="TS")
            SXY = chainp.tile([128, 2, SR, WP], bf16, name="SXY")

            w1sqb = wtsp.tile([128, ST, 2, HIDDEN], bf16, name="w1sqb")
            w1xy8 = wtsp.tile([128, 2, 2, HIDDEN], fp8, name="w1xy8")
            w2pk = wtsp.tile([128, 2, HIDDEN], fp8, name="w2pk")
            w3tp = wtsp.tile([HIDDEN, CB], bf16, name="w3tp")

            fts = {}

            def get_ft(g):
                if g not in fts:
                    ftt = firep.tile([128, 32, W], bf16, name="ft")
                    nc.sync.dma_start(out=ftt[:], in_=fireS_d[g])
                    fts[g] = ftt
                return fts[g]

            # startup DMA order: step-0 first-rowblock deps land first so
            # the prologue chain/masks/s-taps start as early as possible
            nc.sync.dma_start(out=S[0][:, 0:11], in_=s0_d[:, 0:11])
            get_ft(0)
            nc.sync.dma_start(out=w1sqb[:, 0:1], in_=w1sqb_d[:, 0:1])
            nc.sync.dma_start(out=w1xy8[:], in_=w1xy8_d[:])
            nc.sync.dma_start(out=w2pk[:], in_=w2pk_d[:])
            nc.sync.dma_start(out=w3tp[:], in_=w3tp_d[:])
            nc.sync.dma_start(out=S[0][:, 11:SR], in_=s0_d[:, 11:SR])
            if ST > 1:
                nc.sync.dma_start(out=w1sqb[:, 1:ST], in_=w1sqb_d[:, 1:ST])
            nc.sync.dma_start(out=S[1][:], in_=s0_d[:])

            # chain part covering sx/sy strip rows a..b (1-based interior)
            # fast_first=(xn, ft): reorder as P,C,S2,sy,R,sx and emit the
            # sy-plane mask on Pool right after sy so it overlaps the DVE
            # x-path; the sx-plane mask then runs on DVE (critical path).
            def emit_chain_part(g, a, b, fast_first=None):
                """sx/sy for interior strip rows a..b (inclusive, 1-based).
                P=(1+zv)S rows a-1..b; C=(1+zh)P same rows; R=(1+zv)C rows
                a-1..b-1; S2=(1+zh)C rows a-1..b;
                sx(r,x)=R(r-1,x)-R(r-1,x-1); sy(r,x)=S2(r,x-1)-S2(r-1,x-1)."""
                cur = S[g % 2]
                pa = a - 1
                nc.vector.tensor_tensor(
                    out=TP[:, pa:b + 1, 1:67], op=ADD,
                    in0=cur[:, pa:b + 1, 1:67], in1=cur[:, pa + 1:b + 2, 1:67])
                nc.vector.tensor_tensor(
                    out=TC[:, pa:b + 1, 1:66], op=ADD,
                    in0=TP[:, pa:b + 1, 1:66], in1=TP[:, pa:b + 1, 2:67])

                def y_path():
                    nc.vector.tensor_tensor(
                        out=TS[:, pa:b + 1, 1:65], op=ADD,
                        in0=TC[:, pa:b + 1, 1:65], in1=TC[:, pa:b + 1, 2:66])
                    nc.vector.tensor_tensor(
                        out=SXY[:, 1, a:b + 1, CL:CL + W], op=SUB,
                        in0=TS[:, a:b + 1, 1:65], in1=TS[:, pa:b, 1:65])

                def x_path():
                    nc.vector.tensor_tensor(
                        out=TR[:, pa:b, 1:66], op=ADD,
                        in0=TC[:, pa:b, 1:66], in1=TC[:, a:b + 1, 1:66])
                    nc.vector.tensor_tensor(
                        out=SXY[:, 0, a:b + 1, CL:CL + W], op=SUB,
                        in0=TR[:, pa:b, CL:CL + W], in1=TR[:, pa:b, 1:65])

                if fast_first is None:
                    x_path()
                    y_path()
                    return
                xn, ft = fast_first
                y_path()
                nc.gpsimd.tensor_mul(
                    out=xn[:, 1, pa:b, :],
                    in0=SXY[:, 1, a:b + 1, CL:CL + W],
                    in1=ft[:, pa:b, :])
                x_path()
                nc.vector.tensor_mul(
                    out=xn[:, 0, pa:b, :],
                    in0=SXY[:, 0, a:b + 1, CL:CL + W],
                    in1=ft[:, pa:b, :])

            def emit_masks(g, rb, s16m, sxym, ft):
                """Pool: masked bf16 shadow + masked fp8 sobel planes for
                one row-block (interior rows rb*8+1 .. rb*8+8)."""
                cur = S[g % 2]
                r0 = rb * RPT
                nc.gpsimd.tensor_mul(
                    out=s16m[:, r0:r0 + RPT, :],
                    in0=cur[:, r0 + 1:r0 + RPT + 1, CL:CL + W],
                    in1=ft[:, r0:r0 + RPT, :])
                for pl in range(2):
                    nc.gpsimd.tensor_mul(
                        out=sxym[:, pl, r0:r0 + RPT, :],
                        in0=SXY[:, pl, r0 + 1:r0 + RPT + 1, CL:CL + W],
                        in1=ft[:, r0:r0 + RPT, :])

            def emit_halos(g):
                nxt = S[(g + 1) % 2]
                for i in range(B_LOC):
                    ip = i * 2 * CB
                    nc.gpsimd.tensor_copy(out=nxt[ip + CB:ip + CB + NCH, 0:1, :],
                                          in_=nxt[ip:ip + NCH, 32:33, :])
                    nc.gpsimd.tensor_copy(out=nxt[ip:ip + NCH, 33:34, :],
                                          in_=nxt[ip + CB:ip + CB + NCH, 1:2, :])

            # --- steady-state software pipeline over (step, rb, combo) -----
            # rb processing order rotates by +1 each step so each step's
            # chain part (needing dx of rbs r-1..r+1 of the previous step)
            # is ready exactly when the PE reaches it.
            t_ord = [[(g * _ROT + j) % 4 for j in range(4)] for g in range(steps)]

            def tile_at(idx):
                """Global tile index -> (g, rb, c)."""
                g, r = divmod(idx, NT)
                j, c = divmod(r, 4)
                return g, t_ord[g][j], c

            # per-step tiles for masks (double buffered via pool)
            s16m_t, sxym_t = {}, {}
            chain_done = {}

            def get_masks(g):
                if g not in s16m_t:
                    s16m_t[g] = maskp.tile([128, 32, W], bf16, name="s16m")
                    sxym_t[g] = maskp.tile([128, 2, 32, W], fp8, name="sxym")
                return s16m_t[g], sxym_t[g]

            # warm the PE p-state during the init DMAs
            for _w in range(24):
                wp = ph1.tile([HIDDEN, 2, NPIX], f32, name="h1")
                nc.tensor.matmul(wp[:, 0, 0:HIDDEN], w1sqb[0:HIDDEN, 0, 0, :],
                                 w1sqb[0:HIDDEN, 0, 0, :], start=True, stop=True)

            # prologue for step 0: first rb's chain+masks, then the rest as
            # one span so the PE can start while the span computes
            m0, x0 = get_masks(0)
            rb0 = t_ord[0][0]
            r0, r1 = rb0 * RPT, rb0 * RPT + RPT
            nc.gpsimd.tensor_mul(
                out=m0[:, r0:r1, :],
                in0=S[0][:, r0 + 1:r1 + 1, CL:CL + W],
                in1=get_ft(0)[:, r0:r1, :])
            emit_chain_part(0, rb0 * RPT + 1, rb0 * RPT + RPT,
                            fast_first=(x0, get_ft(0)))
            for rb in (r for r in t_ord[0] if r != rb0):
                emit_chain_part(0, rb * RPT + 1, rb * RPT + RPT)
                emit_masks(0, rb, m0, x0, get_ft(0))

            TOT = steps * NT
            pend_h1 = {}     # idx -> (h1 psum, h1s8)
            pend_h2 = {}     # idx -> (h2 psum, h2s)
            pend_dx = {}     # rb-group key -> psum tile

            def emit_stage0(idx):
                g, rb, c = tile_at(idx)
                s16m, sxym = get_masks(g)
                p0 = c * CB
                r0 = rb * RPT
                h1 = ph1.tile([HIDDEN, 2, NPIX], f32, name="h1")
                for half in range(2):
                    nc.tensor.matmul(
                        h1[:, half], w1sqb[p0:p0 + NCH + 1, g, half, :],
                        s16m[p0:p0 + NCH + 1, r0:r0 + RPT, :],
                        start=True, stop=False, tile_position=(p0, 0))
                    nc.tensor.matmul(
                        h1[:, half], w1xy8[p0:p0 + CB, :, half, :],
                        sxym[p0:p0 + CB, :, r0:r0 + RPT, :],
                        start=False, stop=True, perf_mode=DRM,
                        tile_position=(p0, 0))
                pend_h1[idx] = h1

            def emit_stage1(idx):
                h1 = pend_h1.pop(idx)
                h1s8 = actp.tile([HIDDEN, 2, NPIX], fp8, name="h1s8")
                nc.scalar.activation(
                    out=h1s8[:, :, :].rearrange("p a b -> p (a b)"),
                    in_=h1[:, :, :].rearrange("p a b -> p (a b)"),
                    func=LR, bias=0.0, scale=1.0, alpha=0.01)
                h2 = ph2.tile([HIDDEN, NPIX], f32, name="h2")
                nc.tensor.matmul(h2[:], w2pk[:], h1s8[:], start=True, stop=True,
                                 perf_mode=DRM)
                pend_h2[idx] = h2

            def pos_of(g, rb):
                return t_ord[g].index(rb)

            def emit_stage2(idx):
                g, rb, c = tile_at(idx)
                h2 = pend_h2.pop(idx)
                h2s = actp.tile([HIDDEN, NPIX], bf16, name="h2s")
                if (idx % NT) not in _H2_DVE_SLOTS:
                    nc.scalar.activation(out=h2s[:], in_=h2[:], func=LR,
                                         bias=0.0, scale=1.0, alpha=0.01)
                elif _H2_DVE_FORM == "copy_stt":
                    # psum->bf16 copy, then all-SBUF stt max(x, 0.01x) which
                    # runs in the DVE 2x mode
                    tq = actp.tile([HIDDEN, NPIX], bf16, name="tq")
                    nc.vector.tensor_copy(out=tq[:], in_=h2[:])
                    nc.vector.scalar_tensor_tensor(
                        out=h2s[:], in0=tq[:], scalar=0.01, in1=tq[:],
                        op0=mybir.AluOpType.mult, op1=MAX)
                else:
                    tq = actp.tile([HIDDEN, NPIX], bf16, name="tq")
                    nc.vector.tensor_scalar_mul(out=tq[:], in0=h2[:],
                                                scalar1=0.01)
                    nc.vector.tensor_tensor(out=h2s[:], in0=tq[:], in1=h2[:],
                                            op=MAX)
                key = (g, rb)
                if key not in pend_dx:
                    pend_dx[key] = pdx.tile([128, RPT, W], f32, name="dx")
                dxp = pend_dx[key]
                nc.tensor.matmul(
                    dxp[p0b(c):p0b(c) + CB, :, :].rearrange("p a b -> p (a b)"),
                    w3tp[:], h2s[:], start=True, stop=True,
                    tile_position=(0, p0b(c)))
                if c != 3:
                    return
                cur, nxt = S[g % 2], S[(g + 1) % 2]
                r0 = rb * RPT
                nc.vector.tensor_tensor(
                    out=nxt[:, r0 + 1:r0 + RPT + 1, CL:CL + W], op=ADD,
                    in0=cur[:, r0 + 1:r0 + RPT + 1, CL:CL + W], in1=dxp[:])
                del pend_dx[(g, rb)]
                if g == steps - 1:
                    # ship this row-block as soon as it is final; the
                    # epilogue only reads interior rows 1..32, so halo/pad
                    # rows are never transferred
                    nc.sync.dma_start(
                        out=out_d[:, r0 + 1:r0 + RPT + 1, :],
                        in_=nxt[:, r0 + 1:r0 + RPT + 1, :])
                # emit next-step chain/mask parts whose state-row deps are
                # now satisfied: the masks/chain for step g+1 MUST be emitted
                # (program order) before step g+1's matmuls read them, and
                # after their own dx-add deps; chain part rb' reads S rows
                # 8rb'..8rb'+9 -> needs dx of rb'-1, rb', rb'+1 and halos for
                # the edge parts (halos themselves need dx3+dx0).
                jpos = pos_of(g, rb)
                hpos = max(pos_of(g, 0), pos_of(g, 3))
                if jpos == hpos:
                    emit_halos(g)
                if g + 1 >= steps:
                    return
                gn = g + 1
                mn, xn = get_masks(gn)

                # s16m parts only need their own rb's dx and run on the
                # (otherwise idle) Pool engine, so emit them as soon as
                # ready.  Chain parts (DVE) are emitted once their row
                # neighbours' dx (and halos for edge parts) have landed —
                # the first-consumed part becomes ready one dx-add early,
                # shortening the next step's restart; the rest follow after
                # the last dx-add in consumption order.
                for r in range(4):
                    if pos_of(g, r) == jpos:
                        r0, r1 = r * RPT, r * RPT + RPT
                        nc.gpsimd.tensor_mul(
                            out=mn[:, r0:r1, :],
                            in0=S[gn % 2][:, r0 + 1:r1 + 1, CL:CL + W],
                            in1=get_ft(gn)[:, r0:r1, :])

                def chain_pos(rbn):
                    need = [rbn] + [r for r in (rbn - 1, rbn + 1) if 0 <= r < 4]
                    cp = max(pos_of(g, r) for r in need)
                    if rbn in (0, 3):
                        cp = max(cp, hpos)
                    return cp

                # NOTE: the first-consumed part MUST be emitted at its
                # readiness site (jpos<=2): the emission stagger runs the
                # next step's first two stage0s before stage2 of this
                # step's last tile, so waiting for jpos==3 would emit the
                # mask writers after their readers (= race on hardware).
                done = chain_done.setdefault(gn, set())
                for r in t_ord[gn]:
                    if r in done or chain_pos(r) > jpos:
                        continue
                    done.add(r)
                    if _SXYM_FIRST_ON_DVE and r in t_ord[gn][:_NFAST]:
                        emit_chain_part(gn, r * RPT + 1, r * RPT + RPT,
                                        fast_first=(xn, get_ft(gn)))
                        continue
                    emit_chain_part(gn, r * RPT + 1, r * RPT + RPT)
                    r0, r1 = r * RPT, r * RPT + RPT
                    for pl in range(2):
                        nc.gpsimd.tensor_mul(
                            out=xn[:, pl, r0:r1, :],
                            in0=SXY[:, pl, r0 + 1:r1 + 1, CL:CL + W],
                            in1=get_ft(gn)[:, r0:r1, :])

            for idx in range(TOT + 2):
                # stage2 first: the last tiles' h2-act/L3/dx must enter the
                # in-order Act/PE queues before the next step's first-tile
                # work, or the engines head-block on not-yet-ready ops.
                if _ORDER == "210":
                    if 2 <= idx and idx - 2 < TOT:
                        emit_stage2(idx - 2)
                    if 1 <= idx and idx - 1 < TOT:
                        emit_stage1(idx - 1)
                    if idx < TOT:
                        emit_stage0(idx)
                elif _ORDER == "120":
                    if 1 <= idx and idx - 1 < TOT:
                        emit_stage1(idx - 1)
                    if 2 <= idx and idx - 2 < TOT:
                        emit_stage2(idx - 2)
                    if idx < TOT:
                        emit_stage0(idx)
                else:
                    if idx < TOT:
                        emit_stage0(idx)
                    if 1 <= idx and idx - 1 < TOT:
                        emit_stage1(idx - 1)
                    if 2 <= idx and idx - 2 < TOT:
                        emit_stage2(idx - 2)

            # output rows shipped per-rb inside the last step's dx-adds

    orig = nc.to_json_bytes
    nc.to_json_bytes = lambda: _fix_bir_waits(orig())
    return nc


def p0b(c):
    return c * CB


_CACHE = {}


def _get_nc(steps: int):
    if steps not in _CACHE:
        _CACHE[steps] = _build(steps)
    return _CACHE[steps]


def _prep_inputs(x, w1, b1, w2, w3, steps):
    """Host-side input preparation; returns per-core input maps."""
    x = np.asarray(x, np.float32)
    w1 = np.asarray(w1, np.float32)
    b1 = np.asarray(b1, np.float32)
    w2 = np.asarray(w2, np.float32)
    w3 = np.asarray(w3, np.float32)

    hid, fires = _host_rng(steps)
    ST = max(steps, 1)

    bf = ml_dtypes.bfloat16
    f8 = ml_dtypes.float8_e4m3fn

    # full padded state [B, 32, 66, 68]; ones-row at channel slot 29
    state0 = np.zeros((B, CB, H + 2, WP), np.float32)
    state0[:, :NUM_IMG, 1:1 + H, CL:CL + W] = x
    state0[:, NUM_IMG:NUM_IMG + NUM_HID, 1:1 + H, CL:CL + W] = hid
    state0[:, NCH, :, :] = 1.0

    # w1sqb: s-tap weights + per-step (bias + temporal) row, [128, ST, 2, 128]
    w1sqb = np.zeros((128, ST, 2, HIDDEN), np.float32)
    for b0 in (0, 32, 64, 96):
        for half in range(2):
            cs = slice(half * HIDDEN, (half + 1) * HIDDEN)
            w1sqb[b0:b0 + NCH, :, half, :] = w1[cs, 0:NCH].T[:, None, :]
            for t in range(steps):
                be = b1[cs] + w1[cs, 3 * NCH] * (np.float32(t) / np.float32(100.0))
                w1sqb[b0 + NCH, t, half, :] = be
    w1sqb = w1sqb.astype(bf)

    # w1xy8: [128, ktile(sx/sy), half, 128] fp8, /8 sobel scale folded in
    w1xy8 = np.zeros((128, 2, 2, HIDDEN), np.float32)
    for b0 in (0, 32, 64, 96):
        for half in range(2):
            cs = slice(half * HIDDEN, (half + 1) * HIDDEN)
            w1xy8[b0:b0 + NCH, 0, half, :] = w1[cs, NCH:2 * NCH].T / 8.0
            w1xy8[b0:b0 + NCH, 1, half, :] = w1[cs, 2 * NCH:3 * NCH].T / 8.0
    w1xy8 = w1xy8.astype(f8)

    # w2pk: [p, ktile, m] with ktile j holding hidden dims j*128+p
    w2pk = np.zeros((128, 2, HIDDEN), np.float32)
    for j in range(2):
        w2pk[:, j, :] = w2[:, j * HIDDEN:(j + 1) * HIDDEN].T
    w2pk = w2pk.astype(f8)

    # w3tp: [128, 32] with cols 29..31 zero; image channels immutable
    w3tp = np.zeros((HIDDEN, CB), np.float32)
    w3tp[:, :NCH] = w3.T
    w3tp[:, :NUM_IMG] = 0.0
    w3tp = w3tp.astype(bf)

    in_maps = []
    for cidx in range(N_CORES):
        imgs = slice(cidx * B_LOC, (cidx + 1) * B_LOC)
        sc = state0[imgs]                      # [B_LOC, 32, 66, 68]
        s0 = np.stack([sc[:, :, 0:SR, :], sc[:, :, 32:32 + SR, :]], axis=1)
        s0 = s0.reshape(B_LOC * 2 * CB, SR, WP)
        # fireS in strip layout: partition p -> (img, half) block's fire rows
        f = fires[:ST, imgs]                   # [ST, B_LOC, H, W]
        fS = np.zeros((ST, 128, 32, W), np.float32)
        for i in range(B_LOC):
            for hf in range(2):
                blk = f[:, i, hf * 32:hf * 32 + 32, :]      # [ST, 32, W]
                p0 = i * 2 * CB + hf * CB
                fS[:, p0:p0 + CB] = blk[:, None]
        in_maps.append({
            "s0": np.ascontiguousarray(s0),
            "fireS": np.ascontiguousarray(fS).astype(bf),
            "w1sqb": w1sqb, "w1xy8": w1xy8,
            "w2pk": w2pk, "w3tp": w3tp,
        })
    return in_maps


def _softmax(x):
    m = x.max(axis=-1, keepdims=True)
    e = np.exp(x - m)
    return e / e.sum(axis=-1, keepdims=True)


def _epilogue(results):
    logits = np.zeros((B, NUM_OUT), np.float32)
    for c, res in enumerate(results):
        so = res["sout"].reshape(B_LOC, 2, CB, SR, WP)
        cls = so[:, :, NUM_IMG + NUM_HID:NCH, 1:33, CL:CL + W]
        logits[c * B_LOC:(c + 1) * B_LOC] = cls.mean(axis=(1, 3, 4))
    return _softmax(logits).astype(np.float32)


def _run(trace=False, _in_maps=None, **inputs):
    from concourse.bass_utils import run_bass_kernel_spmd
    steps = int(inputs["steps"])
    if steps == 0:
        return _softmax(np.zeros((B, NUM_OUT), np.float32)), None
    in_maps = _in_maps
    if in_maps is None:
        in_maps = _prep_inputs(inputs["x"], inputs["w1"], inputs["b1"],
                               inputs["w2"], inputs["w3"], steps)
    nc = _get_nc(steps)
    r = run_bass_kernel_spmd(nc, in_maps, core_ids=list(range(N_CORES)),
                             trace=trace)
    return _epilogue(r.results), r.exec_time_ns


def predicted_exec_ns(steps: int = 20) -> float:
    """Cost-model (TimelineSim) estimate of on-device execution time for the
    whole job (all cores run the same program in parallel)."""
    from concourse.timeline_sim import TimelineSim
    nc = _build(int(steps))
    return TimelineSim(nc, trace=False).simulate()


def kernel(**inputs) -> np.ndarray:
    out, _ = _run(trace=False, **inputs)
    return out
